# revision 1
# baseline (speedup 1.0000x reference)
"""Trainium2 Bass kernel for nn_NewAttentionBlock (sparse_attention).

Joint softmax attention over a large masked "prior" KV block (S=4096) plus a
small "active" KV block (S=16), for B=8, H=16, Q=16, D=256, fp32.

Sharding: heads are split across the 8 NeuronCores (2 heads/core, tensor
parallel, no cross-core communication).  Each core processes its 16 (b,h)
pairs fully independently.

Per-(b,h) dataflow on a core (all matmuls in float32r on the PE):
  - Q, K_active are transposed on the PE (via identity matmul) to get the
    contraction dim (D) onto partitions.
  - K_prior streams in as [128, 8, 256] tiles (1 MiB DMAs); each 128-row
    s-tile is PE-transposed into K^T chunks [128(d), 512(s)] in SBUF.
  - scores[16, 512] chunks accumulate in PSUM (2 matmuls over the two
    128-halves of D), then ScalarE applies exp(SCALE*s) writing E to SBUF
    while accumulating the per-row sum (softmax denominator) for free.
  - E chunks are PE-transposed to P^T [s, q] and used as the stationary
    operand of the PV matmul against V tiles in natural [s, d] layout,
    accumulating attn_raw[16, 256] in PSUM across all 32 s-tiles + active.
  - The output is attn_raw * (1/denom) via a per-partition tensor_scalar.
The softmax max-subtraction is skipped: scaled scores are ~N(0,1) here so
exp() cannot overflow, and the result is mathematically identical.
prior_mask is all-ones per the problem spec; a numpy fallback handles the
(never expected) general case.
"""

import numpy as np

import concourse.bacc as bacc
import concourse.mybir as mybir
import concourse.tile as tile
from concourse.bass_utils import run_bass_kernel_spmd
from concourse.masks import make_identity

B, H, QL, SP, D = 8, 16, 16, 4096, 256
SCALE = float(D) ** -0.5
N_CORES = 8
HPC = H // N_CORES          # heads per core
NP = B * HPC                # (b,h) pairs per core = 16
ST = 128                    # s-tile size (rows per PE transpose)
CHUNK = 512                 # score-chunk (columns per PSUM score tile)
NCH = SP // CHUNK           # 8 chunks / pair
TPC = CHUNK // ST           # 4 s-tiles per chunk
KDMA = 2048                 # K rows per DMA (2 MiB)
VDMA = 1024                 # V rows per DMA (1 MiB)
NKD = SP // KDMA            # K loads per pair
NVD = SP // VDMA            # V loads per pair

F32 = mybir.dt.float32
F32R = mybir.dt.float32r
EXP = mybir.ActivationFunctionType.Exp

_compiled = None


def _build(loop_n=None):
    nc = bacc.Bacc(
        "TRN2",
        target_bir_lowering=False,
        debug=False,
        num_devices=N_CORES,
    )
    q_d = nc.dram_tensor("q", [NP, QL, D], F32, kind="ExternalInput").ap()
    kp_d = nc.dram_tensor("kp", [NP, SP, D], F32, kind="ExternalInput").ap()
    vp_d = nc.dram_tensor("vp", [NP, SP, D], F32, kind="ExternalInput").ap()
    ka_d = nc.dram_tensor("ka", [NP, QL, D], F32, kind="ExternalInput").ap()
    va_d = nc.dram_tensor("va", [NP, QL, D], F32, kind="ExternalInput").ap()
    out_d = nc.dram_tensor("out", [NP, QL, D], F32, kind="ExternalOutput").ap()

    with tile.TileContext(nc) as tc:
        with (
            tc.tile_pool(name="const", bufs=1) as constp,
            tc.tile_pool(name="kraw", bufs=3) as krawp,
            tc.tile_pool(name="vraw", bufs=7) as vrawp,
            tc.tile_pool(name="ktsb", bufs=6) as ktsbp,
            tc.tile_pool(name="esb", bufs=4) as esbp,
            tc.tile_pool(name="ptsb", bufs=4) as ptsbp,
            tc.tile_pool(name="small", bufs=6) as smallp,
            tc.tile_pool(name="qt", bufs=3) as qtp,
            tc.tile_pool(name="stat", bufs=3) as statp,
            tc.tile_pool(name="osb", bufs=3) as osbp,
            tc.tile_pool(name="ps_kt", bufs=4, space="PSUM") as ps_kt,
            tc.tile_pool(name="ps_s", bufs=2, space="PSUM") as ps_s,
            tc.tile_pool(name="ps_pt", bufs=1, space="PSUM") as ps_pt,
            tc.tile_pool(name="ps_pv", bufs=1, space="PSUM") as ps_pv,
        ):
            ident = constp.tile([128, 128], F32)
            make_identity(nc, ident)

            import contextlib
            loop_cm = (tc.For_i(0, loop_n, 1) if loop_n is not None
                       else contextlib.nullcontext())
            with loop_cm:
              for p in range(NP):
                  # ---- small loads ----------------------------------------
                  q_sb = smallp.tile([QL, D], F32, tag="q")
                  nc.sync.dma_start(out=q_sb, in_=q_d[p])
                  ka_sb = smallp.tile([QL, D], F32, tag="ka")
                  nc.scalar.dma_start(out=ka_sb, in_=ka_d[p])
                  va_sb = smallp.tile([QL, D], F32R, tag="va")
                  nc.scalar.dma_start(out=va_sb, in_=va_d[p].bitcast(F32R))

                  # ---- Q^T / K_active^T  [128, 2*16] ----------------------
                  qt_ps = ps_s.tile([128, 2 * QL], F32, tag="s")
                  kat_ps = ps_s.tile([128, 2 * QL], F32, tag="s")
                  for h in range(2):
                      nc.tensor.transpose(
                          qt_ps[:, h * QL:(h + 1) * QL],
                          q_sb[:, h * 128:(h + 1) * 128],
                          ident[:QL, :QL],
                      )
                      nc.tensor.transpose(
                          kat_ps[:, h * QL:(h + 1) * QL],
                          ka_sb[:, h * 128:(h + 1) * 128],
                          ident[:QL, :QL],
                      )
                  qt_sb = qtp.tile([128, 2 * QL], F32R, tag="qt")
                  nc.any.tensor_copy(qt_sb, qt_ps)
                  kat_sb = qtp.tile([128, 2 * QL], F32R, tag="kat")
                  nc.any.tensor_copy(kat_sb, kat_ps)

                  # ---- active scores + exp + P_active^T -------------------
                  dsum = statp.tile([QL, NCH + 1], F32, tag="dsum")
                  sa_ps = ps_s.tile([QL, QL], F32, tag="s")
                  nc.tensor.matmul(
                      sa_ps, qt_sb[:, 0:QL], kat_sb[:, 0:QL],
                      start=True, stop=False,
                  )
                  nc.tensor.matmul(
                      sa_ps, qt_sb[:, QL:2 * QL], kat_sb[:, QL:2 * QL],
                      start=False, stop=True,
                  )
                  ea_sb = esbp.tile([QL, QL], F32, tag="ea")
                  nc.scalar.activation(
                      ea_sb, sa_ps, EXP, scale=SCALE,
                      accum_out=dsum[:, NCH:NCH + 1],
                  )
                  pta_ps = ps_s.tile([QL, QL], F32, tag="s")
                  nc.tensor.transpose(pta_ps, ea_sb, ident[:QL, :QL])
                  pta_sb = qtp.tile([QL, QL], F32R, tag="pta")
                  nc.any.tensor_copy(pta_sb, pta_ps)

                  # ---- K/V prior streaming loads (K 2 MiB, V 1 MiB) -------
                  kts, vts = [], []
                  for k in range(NKD):
                      kt = krawp.tile([128, KDMA // 128, D], F32, tag="kraw")
                      nc.sync.dma_start(
                          out=kt,
                          in_=kp_d[p, k * KDMA:(k + 1) * KDMA, :].rearrange(
                              "(n q) d -> q n d", q=128),
                      )
                      kts.append(kt)
                  for k in range(NVD):
                      vt = vrawp.tile([128, VDMA // 128, D], F32R, tag="vraw")
                      nc.scalar.dma_start(
                          out=vt,
                          in_=vp_d[p, k * VDMA:(k + 1) * VDMA, :].rearrange(
                              "(n q) d -> q n d", q=128).bitcast(F32R),
                      )
                      vts.append(vt)

                  # ---- prior chunks ---------------------------------------
                  pt_ps = ps_pt.tile([128, NCH * TPC * QL], F32, tag="pt")
                  pv_ps = ps_pv.tile([QL, D], F32, tag="pv")
                  for c in range(NCH):
                      kt_raw = kts[c // (KDMA // CHUNK)]
                      base = (c % (KDMA // CHUNK)) * TPC
                      ktp0 = ps_kt.tile([128, CHUNK], F32, tag="kt")
                      ktp1 = ps_kt.tile([128, CHUNK], F32, tag="kt")
                      for j in range(TPC):
                          nc.tensor.transpose(
                              ktp0[:, j * 128:(j + 1) * 128],
                              kt_raw[:, base + j, 0:128], ident)
                          nc.tensor.transpose(
                              ktp1[:, j * 128:(j + 1) * 128],
                              kt_raw[:, base + j, 128:256], ident)
                      kt0 = ktsbp.tile([128, CHUNK], F32R, tag="kt0")
                      nc.any.tensor_copy(kt0, ktp0)
                      kt1 = ktsbp.tile([128, CHUNK], F32R, tag="kt1")
                      nc.any.tensor_copy(kt1, ktp1)

                      s_ps = ps_s.tile([QL, CHUNK], F32, tag="s")
                      nc.tensor.matmul(
                          s_ps, qt_sb[:, 0:QL], kt0,
                          start=True, stop=False)
                      nc.tensor.matmul(
                          s_ps, qt_sb[:, QL:2 * QL], kt1,
                          start=False, stop=True)

                      e_sb = esbp.tile([QL, CHUNK], F32, tag="e")
                      nc.scalar.activation(
                          e_sb, s_ps, EXP, scale=SCALE,
                          accum_out=dsum[:, c:c + 1],
                      )
                      for j in range(TPC):
                          nc.tensor.transpose(
                              pt_ps[:, c * TPC * QL + j * QL:
                                    c * TPC * QL + (j + 1) * QL],
                              e_sb[:, j * 128:(j + 1) * 128],
                              ident[:QL, :QL],
                          )
                      ptc = ptsbp.tile([128, TPC * QL], F32R, tag="ptc")
                      nc.any.tensor_copy(
                          ptc, pt_ps[:, c * TPC * QL:(c + 1) * TPC * QL])
                      for j in range(TPC):
                          st = c * TPC + j
                          v_raw = vts[st // (VDMA // 128)]
                          nc.tensor.matmul(
                              pv_ps,
                              ptc[:, j * QL:(j + 1) * QL],
                              v_raw[:, st % (VDMA // 128), :],
                              start=(st == 0), stop=False,
                          )
                  # active PV contribution last (closes the accumulation)
                  nc.tensor.matmul(
                      pv_ps, pta_sb, va_sb, start=False, stop=True)

                  # ---- normalize + store ----------------------------------
                  den = statp.tile([QL, 1], F32, tag="den")
                  nc.vector.reduce_sum(
                      out=den, in_=dsum[:, 0:NCH + 1], axis=mybir.AxisListType.X)
                  rec = statp.tile([QL, 1], F32, tag="rec")
                  nc.vector.reciprocal(rec, den)
                  o_sb = osbp.tile([QL, D], F32, tag="o")
                  nc.vector.tensor_scalar_mul(o_sb, pv_ps, rec)
                  nc.gpsimd.dma_start(out=out_d[p], in_=o_sb)

    nc.compile()
    return nc


def _get_compiled():
    global _compiled
    if _compiled is None:
        _compiled = _build()
    return _compiled


def make_in_maps(Q, K_prior, V_prior, K_active, V_active):
    in_maps = []
    for c in range(N_CORES):
        hs = slice(c * HPC, (c + 1) * HPC)
        in_maps.append({
            "q": np.ascontiguousarray(Q[:, hs]).reshape(NP, QL, D),
            "kp": np.ascontiguousarray(K_prior[:, hs]).reshape(NP, SP, D),
            "vp": np.ascontiguousarray(V_prior[:, hs]).reshape(NP, SP, D),
            "ka": np.ascontiguousarray(K_active[:, hs]).reshape(NP, QL, D),
            "va": np.ascontiguousarray(V_active[:, hs]).reshape(NP, QL, D),
        })
    return in_maps


def gather_out(per_core_outs):
    full = np.stack(per_core_outs, axis=0).reshape(N_CORES, B, HPC, QL, D)
    return np.ascontiguousarray(
        full.transpose(1, 0, 2, 3, 4).reshape(B, H, QL, D))


def _numpy_fallback(Q, K_prior, V_prior, K_active, V_active, prior_mask):
    ps = np.einsum("bhqd,bhkd->bhqk", Q, K_prior) * SCALE
    as_ = np.einsum("bhqd,bhkd->bhqk", Q, K_active) * SCALE
    neg = np.finfo(np.float32).min
    ps = np.where(prior_mask, ps, neg)
    m = np.maximum(ps.max(-1, keepdims=True), as_.max(-1, keepdims=True))
    ep = np.exp(ps - m)
    ea = np.exp(as_ - m)
    den = ep.sum(-1, keepdims=True) + ea.sum(-1, keepdims=True)
    return (np.einsum("bhqk,bhkd->bhqd", (ep / den).astype(np.float32), V_prior)
            + np.einsum("bhqk,bhkd->bhqd", (ea / den).astype(np.float32),
                        V_active)).astype(np.float32)


def kernel(**inputs):
    Q = np.asarray(inputs["Q"], dtype=np.float32)
    K_prior = np.asarray(inputs["K_prior"], dtype=np.float32)
    V_prior = np.asarray(inputs["V_prior"], dtype=np.float32)
    K_active = np.asarray(inputs["K_active"], dtype=np.float32)
    V_active = np.asarray(inputs["V_active"], dtype=np.float32)
    prior_mask = np.asarray(inputs["prior_mask"])

    if not prior_mask.all():
        # Spec guarantees an all-ones mask; general masks take the slow path.
        return _numpy_fallback(Q, K_prior, V_prior, K_active, V_active,
                               prior_mask)

    nc = _get_compiled()
    res = run_bass_kernel_spmd(
        nc,
        make_in_maps(Q, K_prior, V_prior, K_active, V_active),
        core_ids=list(range(N_CORES)),
    )
    return gather_out([res.results[c]["out"] for c in range(N_CORES)])



# revision 2
# speedup vs baseline: 2.0747x; 2.0747x over previous
"""Trainium2 Bass kernel for nn_NewAttentionBlock (sparse_attention).

Joint softmax attention over a large all-ones-masked "prior" KV block
(S=4096) plus a small "active" KV block (S=16), for B=8, H=16, Q=16,
D=256.  Heads are split across the 8 NeuronCores (2 heads/core, tensor
parallel, no cross-core communication); each core processes its 16 (b,h)
pairs independently.

The problem is HBM-bandwidth bound (K_prior/V_prior are streamed once and
never reused), so the kernel:

  - casts all inputs to bf16 on the host (rel-err budget is 2e-2; bf16
    contributes ~3e-3), halving HBM traffic vs fp32;
  - pre-packs all layouts on the host so the device does ZERO transposes:
      * K^T tiles [d, s] so scores are computed directly in S^T
        orientation: S^T[s,q] = sum_d K^T[d,s] Q^T[d,q] with K^T slices as
        the PE stationary operand (full 128-col weights -> FWL);
      * V in s-partition-major tiles with a ones-column appended, so the
        PV matmul accumulates both attn_raw[q,d] and the softmax
        denominator sum_s P[q,s] in one PSUM region;
  - per (b,h) pair: 64 score matmuls fill one PSUM bank with S^T
    [128, 32*16], one ScalarE exp over the whole bank writes P^T bf16 to
    SBUF, 32+1 PV matmuls accumulate [16, 257], then VectorE normalizes
    by the reciprocal of column 256.
  - PV for pair p-1 is issued after the score matmuls of pair p, so the
    PE never waits on ScalarE's exp.

The softmax max-subtraction is skipped: scaled scores are ~N(0,1), so
exp() cannot overflow, and with an all-ones mask the result is
mathematically identical.  A numpy fallback handles the (never expected)
general-mask case.
"""

import numpy as np
import ml_dtypes

import concourse.bacc as bacc
import concourse.mybir as mybir
import concourse.tile as tile
from concourse.bass_utils import run_bass_kernel_spmd

B, H, QL, SP, D = 8, 16, 16, 4096, 256
SCALE = float(D) ** -0.5
N_CORES = 8
HPC = H // N_CORES          # heads per core
NP = B * HPC                # (b,h) pairs per core = 16
ST = 128                    # s-tile size (PSUM partition dim)
NT = SP // ST               # 32 s-tiles per pair
DH = D // 128               # 2 contraction halves

F32 = mybir.dt.float32
BF16 = mybir.dt.bfloat16
BF16NP = ml_dtypes.bfloat16
EXP = mybir.ActivationFunctionType.Exp

_compiled = None


def _build(loop_n=None):
    nc = bacc.Bacc(
        "TRN2",
        target_bir_lowering=False,
        debug=False,
        num_devices=N_CORES,
    )
    # host-prepacked layouts (see make_in_maps):
    #   kp: [NP, 128, 2, 4096]  K^T tiles: [d%128, d//128, s]
    #   vp: [NP, 128, 32, 257]  V tiles [s%128, s//128, d] + ones col
    #   qk: [NP, 128, 64]       Q^T h0|h1, K_active^T h0|h1 (16 cols each)
    #   va: [NP, 16, 257]       V_active + ones col
    kp_d = nc.dram_tensor("kp", [NP, 128, DH, SP], BF16, kind="ExternalInput").ap()
    vp_d = nc.dram_tensor("vp", [NP, 128, NT, D + 1], BF16, kind="ExternalInput").ap()
    qk_d = nc.dram_tensor("qk", [NP, 128, 4 * QL], BF16, kind="ExternalInput").ap()
    va_d = nc.dram_tensor("va", [NP, QL, D + 1], BF16, kind="ExternalInput").ap()
    out_d = nc.dram_tensor("out", [NP, QL, D], F32, kind="ExternalOutput").ap()

    with tile.TileContext(nc) as tc:
        with (
            tc.tile_pool(name="kt", bufs=3) as ktp,
            tc.tile_pool(name="vt", bufs=3) as vtp,
            tc.tile_pool(name="qk", bufs=3) as qkp,
            tc.tile_pool(name="va", bufs=3) as vap,
            tc.tile_pool(name="pt", bufs=2) as ptp,
            tc.tile_pool(name="pa", bufs=2) as pap,
            tc.tile_pool(name="stat", bufs=3) as statp,
            tc.tile_pool(name="osb", bufs=3) as osbp,
            tc.tile_pool(name="ps_s", bufs=2, space="PSUM") as ps_s,
            tc.tile_pool(name="ps_a", bufs=2, space="PSUM") as ps_a,
            tc.tile_pool(name="ps_pv", bufs=2, space="PSUM") as ps_pv,
        ):
            import contextlib
            loop_cm = (tc.For_i(0, loop_n, 1) if loop_n is not None
                       else contextlib.nullcontext())
            with loop_cm:
                # one software-pipeline stage: PV for pair p-1 is issued
                # after the score matmuls of pair p.
                pend = [None]

                def flush_pv():
                    if pend[0] is None:
                        return
                    p, pt_sb, pa_sb, vt, va_sb = pend[0]
                    pend[0] = None
                    pv_ps = ps_pv.tile([QL, D + 1], F32, tag="pv")
                    for t in range(NT):
                        nc.tensor.matmul(
                            pv_ps,
                            pt_sb[:, t * QL:(t + 1) * QL],
                            vt[:, t, :],
                            start=(t == 0), stop=False,
                        )
                    nc.tensor.matmul(pv_ps, pa_sb, va_sb,
                                     start=False, stop=True)
                    rec = statp.tile([QL, 1], F32, tag="rec")
                    nc.vector.reciprocal(rec, pv_ps[:, D:D + 1])
                    o_sb = osbp.tile([QL, D], F32, tag="o")
                    nc.vector.tensor_scalar_mul(o_sb, pv_ps[:, 0:D], rec)
                    nc.gpsimd.dma_start(out=out_d[p], in_=o_sb)

                for p in range(NP):
                    # ---- streaming loads (all layouts device-ready) ----
                    kt = ktp.tile([128, DH, SP], BF16, tag="kt")
                    nc.sync.dma_start(out=kt, in_=kp_d[p])
                    qk_sb = qkp.tile([128, 4 * QL], BF16, tag="qk")
                    nc.sync.dma_start(out=qk_sb, in_=qk_d[p])
                    vt = vtp.tile([128, NT, D + 1], BF16, tag="vt")
                    nc.scalar.dma_start(out=vt, in_=vp_d[p])
                    va_sb = vap.tile([QL, D + 1], BF16, tag="va")
                    nc.scalar.dma_start(out=va_sb, in_=va_d[p])

                    # ---- prior scores, S^T orientation ------------------
                    # s_ps[:, t*16:(t+1)*16] = K^T-tile(t)^T-contracted Q^T
                    s_ps = ps_s.tile([128, NT * QL], F32, tag="s")
                    for t in range(NT):
                        for h in range(DH):
                            nc.tensor.matmul(
                                s_ps[:, t * QL:(t + 1) * QL],
                                kt[:, h, t * 128:(t + 1) * 128],
                                qk_sb[:, h * QL:(h + 1) * QL],
                                start=(h == 0), stop=(h == DH - 1),
                            )
                    # active scores S_a^T [16, 16]
                    sa_ps = ps_a.tile([QL, QL], F32, tag="sa")
                    for h in range(DH):
                        nc.tensor.matmul(
                            sa_ps,
                            qk_sb[:, (2 + h) * QL:(3 + h) * QL],
                            qk_sb[:, h * QL:(h + 1) * QL],
                            start=(h == 0), stop=(h == DH - 1),
                        )

                    # ---- PV for the previous pair (PE never idles) ------
                    flush_pv()

                    # ---- exp -> P^T (bf16) ------------------------------
                    pt_sb = ptp.tile([128, NT * QL], BF16, tag="pt")
                    nc.scalar.activation(pt_sb, s_ps, EXP, scale=SCALE)
                    pa_sb = pap.tile([QL, QL], BF16, tag="pa")
                    nc.scalar.activation(pa_sb, sa_ps, EXP, scale=SCALE)

                    pend[0] = (p, pt_sb, pa_sb, vt, va_sb)

                flush_pv()

    nc.compile()
    return nc


def _get_compiled():
    global _compiled
    if _compiled is None:
        _compiled = _build()
    return _compiled


def _pack_core(Q, K_prior, V_prior, K_active, V_active):
    """Pack one core's [NP, ...] f32 slices into device layouts (bf16)."""
    # K^T tiles: kp[p, dd, h, s] = K[p, s, h*128+dd]
    kp = np.ascontiguousarray(
        K_prior.astype(BF16NP).reshape(NP, SP, DH, 128).transpose(0, 3, 2, 1))
    # V s-partition-major + ones column: vp[p, q, n, d] = V[p, n*128+q, d]
    vp = np.empty((NP, 128, NT, D + 1), dtype=BF16NP)
    vp[..., :D] = V_prior.astype(BF16NP).reshape(
        NP, NT, 128, D).transpose(0, 2, 1, 3)
    vp[..., D] = np.asarray(1.0, dtype=BF16NP)
    # Q^T / K_active^T packed: qk[p, dd, h*16+q] = Q[p, q, h*128+dd]
    qk = np.empty((NP, 128, 4 * QL), dtype=BF16NP)
    qk[:, :, 0:2 * QL] = Q.astype(BF16NP).reshape(
        NP, QL, DH, 128).transpose(0, 3, 2, 1).reshape(NP, 128, 2 * QL)
    qk[:, :, 2 * QL:] = K_active.astype(BF16NP).reshape(
        NP, QL, DH, 128).transpose(0, 3, 2, 1).reshape(NP, 128, 2 * QL)
    va = np.empty((NP, QL, D + 1), dtype=BF16NP)
    va[..., :D] = V_active.astype(BF16NP)
    va[..., D] = np.asarray(1.0, dtype=BF16NP)
    return {"kp": kp, "vp": vp, "qk": qk, "va": va}


def make_in_maps(Q, K_prior, V_prior, K_active, V_active):
    in_maps = []
    for c in range(N_CORES):
        hs = slice(c * HPC, (c + 1) * HPC)
        in_maps.append(_pack_core(
            np.ascontiguousarray(Q[:, hs]).reshape(NP, QL, D),
            np.ascontiguousarray(K_prior[:, hs]).reshape(NP, SP, D),
            np.ascontiguousarray(V_prior[:, hs]).reshape(NP, SP, D),
            np.ascontiguousarray(K_active[:, hs]).reshape(NP, QL, D),
            np.ascontiguousarray(V_active[:, hs]).reshape(NP, QL, D),
        ))
    return in_maps


def gather_out(per_core_outs):
    full = np.stack(per_core_outs, axis=0).reshape(N_CORES, B, HPC, QL, D)
    return np.ascontiguousarray(
        full.transpose(1, 0, 2, 3, 4).reshape(B, H, QL, D))


def _numpy_fallback(Q, K_prior, V_prior, K_active, V_active, prior_mask):
    ps = np.einsum("bhqd,bhkd->bhqk", Q, K_prior) * SCALE
    as_ = np.einsum("bhqd,bhkd->bhqk", Q, K_active) * SCALE
    neg = np.finfo(np.float32).min
    ps = np.where(prior_mask, ps, neg)
    m = np.maximum(ps.max(-1, keepdims=True), as_.max(-1, keepdims=True))
    ep = np.exp(ps - m)
    ea = np.exp(as_ - m)
    den = ep.sum(-1, keepdims=True) + ea.sum(-1, keepdims=True)
    return (np.einsum("bhqk,bhkd->bhqd", (ep / den).astype(np.float32), V_prior)
            + np.einsum("bhqk,bhkd->bhqd", (ea / den).astype(np.float32),
                        V_active)).astype(np.float32)


def kernel(**inputs):
    Q = np.asarray(inputs["Q"], dtype=np.float32)
    K_prior = np.asarray(inputs["K_prior"], dtype=np.float32)
    V_prior = np.asarray(inputs["V_prior"], dtype=np.float32)
    K_active = np.asarray(inputs["K_active"], dtype=np.float32)
    V_active = np.asarray(inputs["V_active"], dtype=np.float32)
    prior_mask = np.asarray(inputs["prior_mask"])

    if not prior_mask.all():
        # Spec guarantees an all-ones mask; general masks take the slow path.
        return _numpy_fallback(Q, K_prior, V_prior, K_active, V_active,
                               prior_mask)

    nc = _get_compiled()
    res = run_bass_kernel_spmd(
        nc,
        make_in_maps(Q, K_prior, V_prior, K_active, V_active),
        core_ids=list(range(N_CORES)),
    )
    return gather_out([res.results[c]["out"] for c in range(N_CORES)])


# revision 3
# speedup vs baseline: 2.2618x; 1.0902x over previous
"""Trainium2 Bass kernel for nn_NewAttentionBlock (sparse_attention).

Joint softmax attention over a large all-ones-masked "prior" KV block
(S=4096) plus a small "active" KV block (S=16), for B=8, H=16, Q=16,
D=256.  Heads are split across the 8 NeuronCores (2 heads/core, tensor
parallel, no cross-core communication); each core processes its 16 (b,h)
pairs independently.

The problem is HBM-bandwidth bound (K_prior/V_prior are streamed once and
never reused), so the kernel:

  - casts all inputs to bf16 on the host (rel-err budget is 2e-2; bf16
    contributes ~6e-3), halving HBM traffic vs fp32;
  - pre-packs each pair's entire input into ONE partition-major buffer
    kv = [Q^T|K_active^T (64 cols) | K^T tiles (8192) | V tiles+ones
    (8224)], so the device does ZERO transposes and each pair needs just
    two large back-to-back DMAs on one HWDGE ring, pairs alternating
    between the sync and scalar rings (measured at the ~358 GB/s
    HBM-per-core line rate, vs ~325 GB/s for layout-split DMA streams);
  - computes scores directly in S^T orientation: S^T[s,q] =
    sum_d K^T[d,s] Q^T[d,q] with K^T slices as the PE stationary operand
    (full 128-col bf16 weights -> fast weight load);
  - appends a ones-column to V so the PV matmul accumulates both
    attn_raw[q,d] and the softmax denominator sum_s P[q,s] in one PSUM
    region: per pair, 64 score matmuls fill one PSUM bank with S^T
    [128, 32*16], one ScalarE exp over the whole bank writes P^T bf16 to
    SBUF, 32+1 PV matmuls accumulate [16, 257], then VectorE normalizes
    by the reciprocal of column 256;
  - issues PV for pair p-1 after the score matmuls of pair p, so the PE
    never waits on ScalarE's exp.

The softmax max-subtraction is skipped: scaled scores are ~N(0,1), so
exp() cannot overflow, and with an all-ones mask the result is
mathematically identical.  A numpy fallback handles the (never expected)
general-mask case.
"""

import numpy as np
import ml_dtypes

import concourse.bacc as bacc
import concourse.mybir as mybir
import concourse.tile as tile
from concourse.bass_utils import run_bass_kernel_spmd

B, H, QL, SP, D = 8, 16, 16, 4096, 256
SCALE = float(D) ** -0.5
N_CORES = 8
HPC = H // N_CORES          # heads per core
NP = B * HPC                # (b,h) pairs per core = 16
ST = 128                    # s-tile size (PSUM partition dim)
NT = SP // ST               # 32 s-tiles per pair
DH = D // 128               # 2 contraction halves

# column offsets inside the fused per-pair kv buffer [128, KVW]
QK0 = 0                     # Q^T (2x16) | K_active^T (2x16)
KP0 = 4 * QL                # K^T tiles: [h*SP + s]
VP0 = KP0 + DH * SP         # V tiles: [t*(D+1) + d], col D is ones
KVW = VP0 + NT * (D + 1)

F32 = mybir.dt.float32
BF16 = mybir.dt.bfloat16
BF16NP = ml_dtypes.bfloat16
EXP = mybir.ActivationFunctionType.Exp

_compiled = None


def _build(loop_n=None):
    nc = bacc.Bacc(
        "TRN2",
        target_bir_lowering=False,
        debug=False,
        num_devices=N_CORES,
    )
    kv_d = nc.dram_tensor("kv", [NP, 128, KVW], BF16, kind="ExternalInput").ap()
    va_d = nc.dram_tensor("va", [NP, QL, D + 1], BF16, kind="ExternalInput").ap()
    out_d = nc.dram_tensor("out", [NP, QL, D], F32, kind="ExternalOutput").ap()

    with tile.TileContext(nc) as tc:
        with (
            tc.tile_pool(name="kv", bufs=3) as kvp,
            tc.tile_pool(name="va", bufs=3) as vap,
            tc.tile_pool(name="pt", bufs=2) as ptp,
            tc.tile_pool(name="pa", bufs=2) as pap,
            tc.tile_pool(name="stat", bufs=3) as statp,
            tc.tile_pool(name="osb", bufs=3) as osbp,
            tc.tile_pool(name="ps_s", bufs=2, space="PSUM") as ps_s,
            tc.tile_pool(name="ps_a", bufs=2, space="PSUM") as ps_a,
            tc.tile_pool(name="ps_pv", bufs=2, space="PSUM") as ps_pv,
        ):
            import contextlib
            loop_cm = (tc.For_i(0, loop_n, 1) if loop_n is not None
                       else contextlib.nullcontext())
            with loop_cm:
                # one software-pipeline stage: PV for pair p-1 is issued
                # after the score matmuls of pair p.
                pend = [None]

                def flush_pv():
                    if pend[0] is None:
                        return
                    p, pt_sb, pa_sb, kv, va_sb = pend[0]
                    pend[0] = None
                    pv_ps = ps_pv.tile([QL, D + 1], F32, tag="pv")
                    for t in range(NT):
                        nc.tensor.matmul(
                            pv_ps,
                            pt_sb[:, t * QL:(t + 1) * QL],
                            kv[:, VP0 + t * (D + 1):VP0 + (t + 1) * (D + 1)],
                            start=(t == 0), stop=False,
                        )
                    nc.tensor.matmul(pv_ps, pa_sb, va_sb,
                                     start=False, stop=True)
                    rec = statp.tile([QL, 1], F32, tag="rec")
                    nc.vector.reciprocal(rec, pv_ps[:, D:D + 1])
                    o_sb = osbp.tile([QL, D], F32, tag="o")
                    nc.vector.tensor_scalar_mul(o_sb, pv_ps[:, 0:D], rec)
                    nc.gpsimd.dma_start(out=out_d[p], in_=o_sb)

                for p in range(NP):
                    # ---- fused streaming loads, pair-parity HWDGE ring --
                    eng = nc.sync if p % 2 == 0 else nc.scalar
                    kv = kvp.tile([128, KVW], BF16, tag="kv")
                    # chunk 1: qk + K^T (scores inputs); chunk 2: V tiles
                    eng.dma_start(out=kv[:, 0:VP0], in_=kv_d[p, :, 0:VP0])
                    eng.dma_start(out=kv[:, VP0:KVW], in_=kv_d[p, :, VP0:KVW])
                    va_sb = vap.tile([QL, D + 1], BF16, tag="va")
                    eng.dma_start(out=va_sb, in_=va_d[p])

                    # ---- prior scores, S^T orientation ------------------
                    s_ps = ps_s.tile([128, NT * QL], F32, tag="s")
                    for t in range(NT):
                        for h in range(DH):
                            nc.tensor.matmul(
                                s_ps[:, t * QL:(t + 1) * QL],
                                kv[:, KP0 + h * SP + t * 128:
                                   KP0 + h * SP + (t + 1) * 128],
                                kv[:, h * QL:(h + 1) * QL],
                                start=(h == 0), stop=(h == DH - 1),
                            )
                    # active scores S_a^T [16, 16]
                    sa_ps = ps_a.tile([QL, QL], F32, tag="sa")
                    for h in range(DH):
                        nc.tensor.matmul(
                            sa_ps,
                            kv[:, (2 + h) * QL:(3 + h) * QL],
                            kv[:, h * QL:(h + 1) * QL],
                            start=(h == 0), stop=(h == DH - 1),
                        )

                    # ---- PV for the previous pair (PE never idles) ------
                    flush_pv()

                    # ---- exp -> P^T (bf16) ------------------------------
                    pt_sb = ptp.tile([128, NT * QL], BF16, tag="pt")
                    nc.scalar.activation(pt_sb, s_ps, EXP, scale=SCALE)
                    pa_sb = pap.tile([QL, QL], BF16, tag="pa")
                    nc.scalar.activation(pa_sb, sa_ps, EXP, scale=SCALE)

                    pend[0] = (p, pt_sb, pa_sb, kv, va_sb)

                flush_pv()

    nc.compile()
    return nc


def _get_compiled():
    global _compiled
    if _compiled is None:
        _compiled = _build()
    return _compiled


def _pack_core(Q, K_prior, V_prior, K_active, V_active):
    """Pack one core's [NP, ...] f32 slices into device layouts (bf16)."""
    kv = np.empty((NP, 128, KVW), dtype=BF16NP)
    # Q^T / K_active^T: [dd, h*16+q] = X[q, h*128+dd]
    kv[:, :, QK0:QK0 + 2 * QL] = Q.astype(BF16NP).reshape(
        NP, QL, DH, 128).transpose(0, 3, 2, 1).reshape(NP, 128, 2 * QL)
    kv[:, :, QK0 + 2 * QL:KP0] = K_active.astype(BF16NP).reshape(
        NP, QL, DH, 128).transpose(0, 3, 2, 1).reshape(NP, 128, 2 * QL)
    # K^T tiles: [dd, h*SP+s] = K[s, h*128+dd]
    kv[:, :, KP0:VP0] = K_prior.astype(BF16NP).reshape(
        NP, SP, DH, 128).transpose(0, 3, 2, 1).reshape(NP, 128, DH * SP)
    # V s-partition-major + ones col: [q, t*(D+1)+d] = V[t*128+q, d]
    vt = kv[:, :, VP0:KVW].reshape(NP, 128, NT, D + 1)
    vt[..., :D] = V_prior.astype(BF16NP).reshape(
        NP, NT, 128, D).transpose(0, 2, 1, 3)
    vt[..., D] = np.asarray(1.0, dtype=BF16NP)
    va = np.empty((NP, QL, D + 1), dtype=BF16NP)
    va[..., :D] = V_active.astype(BF16NP)
    va[..., D] = np.asarray(1.0, dtype=BF16NP)
    return {"kv": kv, "va": va}


def make_in_maps(Q, K_prior, V_prior, K_active, V_active):
    in_maps = []
    for c in range(N_CORES):
        hs = slice(c * HPC, (c + 1) * HPC)
        in_maps.append(_pack_core(
            np.ascontiguousarray(Q[:, hs]).reshape(NP, QL, D),
            np.ascontiguousarray(K_prior[:, hs]).reshape(NP, SP, D),
            np.ascontiguousarray(V_prior[:, hs]).reshape(NP, SP, D),
            np.ascontiguousarray(K_active[:, hs]).reshape(NP, QL, D),
            np.ascontiguousarray(V_active[:, hs]).reshape(NP, QL, D),
        ))
    return in_maps


def gather_out(per_core_outs):
    full = np.stack(per_core_outs, axis=0).reshape(N_CORES, B, HPC, QL, D)
    return np.ascontiguousarray(
        full.transpose(1, 0, 2, 3, 4).reshape(B, H, QL, D))


def _numpy_fallback(Q, K_prior, V_prior, K_active, V_active, prior_mask):
    ps = np.einsum("bhqd,bhkd->bhqk", Q, K_prior) * SCALE
    as_ = np.einsum("bhqd,bhkd->bhqk", Q, K_active) * SCALE
    neg = np.finfo(np.float32).min
    ps = np.where(prior_mask, ps, neg)
    m = np.maximum(ps.max(-1, keepdims=True), as_.max(-1, keepdims=True))
    ep = np.exp(ps - m)
    ea = np.exp(as_ - m)
    den = ep.sum(-1, keepdims=True) + ea.sum(-1, keepdims=True)
    return (np.einsum("bhqk,bhkd->bhqd", (ep / den).astype(np.float32), V_prior)
            + np.einsum("bhqk,bhkd->bhqd", (ea / den).astype(np.float32),
                        V_active)).astype(np.float32)


def kernel(**inputs):
    Q = np.asarray(inputs["Q"], dtype=np.float32)
    K_prior = np.asarray(inputs["K_prior"], dtype=np.float32)
    V_prior = np.asarray(inputs["V_prior"], dtype=np.float32)
    K_active = np.asarray(inputs["K_active"], dtype=np.float32)
    V_active = np.asarray(inputs["V_active"], dtype=np.float32)
    prior_mask = np.asarray(inputs["prior_mask"])

    if not prior_mask.all():
        # Spec guarantees an all-ones mask; general masks take the slow path.
        return _numpy_fallback(Q, K_prior, V_prior, K_active, V_active,
                               prior_mask)

    nc = _get_compiled()
    res = run_bass_kernel_spmd(
        nc,
        make_in_maps(Q, K_prior, V_prior, K_active, V_active),
        core_ids=list(range(N_CORES)),
    )
    return gather_out([res.results[c]["out"] for c in range(N_CORES)])
